# revision 18
# baseline (speedup 1.0000x reference)
"""LCNN conv2d kernel for Trainium2 (8 NeuronCores, batch-sharded).

Math: out[b,o,h,w] = sum_d Wmat[o,d] * conv2d(x, dictionary)[b,d,h,w]
where Wmat is the scatter-add of lookup_coefficients into [O, D].

Device strategy (per core, 2 batches), all matmuls bf16 (PE reaches the
2.4 GHz pstate; f32r holds it near 1.2 GHz):
 - stage 1: conv with the D=100 dictionary in 5 accumulating matmuls per
   output tile. Two shifted copies of x pack two kernel taps into the
   128-partition contraction: XA pairs (kh,0)+(kh,1) via a +1 shift,
   XB pairs (0,2)+(1,2) via a +PW shift; only tap (2,2) runs alone.
 - stage 2: [O=256, D=100] channel-mix matmul on the conv result.
 - output staged and stored as bf16 (halves HBM store traffic); host
   upcasts to f32.
"""
import os
import sys

for _p in ("/opt/trn_rl_repo", "/root/.axon_site/_ro/trn_rl_repo"):
    if os.path.isdir(_p) and _p not in sys.path:
        sys.path.insert(0, _p)

import ml_dtypes
import numpy as np
from contextlib import ExitStack

from concourse import bacc, mybir, tile
from concourse.bass_utils import run_bass_kernel_spmd

# problem shapes (hardcoded per contract)
B, CIN, H, W = 16, 64, 96, 96
D, O = 100, 256
NCORES = 8
BPC = B // NCORES          # batches per core
PH, PW = H + 2, W + 2      # zero-padded spatial
F = BPC * PH * PW          # per-partition x extent
R = 4                      # output rows per matmul tile
NT = H // R                # h-tiles per batch
G = 4                      # h-tiles per output-DMA group
NG = NT // G
N = R * W                  # matmul free size (384)
f32 = mybir.dt.float32
bf16 = mybir.dt.bfloat16

_NC_CACHE = {}


def _build():
    nc = bacc.Bacc(None, target_bir_lowering=False, debug=False)
    # host pre-casts to bf16 so loads use the fast no-cast HWDGE path, and
    # pre-builds both 128-partition x planes (plain + shifted halves) so no
    # on-chip shuffling is needed: xa = [x; x shifted +1], xb = [x; x
    # shifted +PW].
    xa = nc.declare_dram_parameter("xa", [128, F], bf16, isOutput=False)
    xb = nc.declare_dram_parameter("xb", [128, F], bf16, isOutput=False)
    wa = nc.declare_dram_parameter("wa", [128, 3 * D], bf16, isOutput=False)
    wb = nc.declare_dram_parameter("wb", [128, 2 * D], bf16, isOutput=False)
    wm = nc.declare_dram_parameter("wm", [D, O], bf16, isOutput=False)
    # output in staging-buffer order (one flat 2D HWDGE store per group;
    # 3D/strided APs would fall back to the ~155 GB/s software DGE); the
    # host un-shuffles for free.
    out = nc.declare_dram_parameter("out", [BPC * NG, 128, 2 * G * N], bf16,
                                    isOutput=True)

    # h-rows covered by each x sub-tile (two overlapping sub-tiles per
    # batch so every conv tile's 6-row window lives in exactly one).
    SOFF = NT // 2 * R         # second sub-tile starts at h=48
    SH = SOFF + 2              # 50 rows each (= PH - SOFF)

    with tile.TileContext(nc) as tc, ExitStack() as ctx:
        sb = ctx.enter_context(tc.tile_pool(name="sb", bufs=1))
        conv1p = ctx.enter_context(tc.tile_pool(name="conv1p", bufs=3))
        stgp = ctx.enter_context(tc.tile_pool(name="stgp", bufs=2))
        pcp = ctx.enter_context(tc.tile_pool(name="pcp", bufs=2, space="PSUM"))
        pop = ctx.enter_context(tc.tile_pool(name="pop", bufs=2, space="PSUM"))

        wa_s = sb.tile([128, 3 * D], bf16)
        wb_s = sb.tile([128, 2 * D], bf16)
        wm_s = sb.tile([D, O], bf16)
        nc.sync.dma_start(wa_s[:], wa[:])
        nc.sync.dma_start(wb_s[:], wb[:])
        nc.sync.dma_start(wm_s[:], wm[:])

        # x planes loaded as 4 sub-tiles each (per batch, upper/lower half
        # heights). Dependency tracking is tile-granular and the DMA
        # engines fair-share bandwidth across ALL pending descriptors, so:
        # only batch 0 is issued upfront (sub-tile 0 first); batch 1 is
        # issued later from inside the compute loop (via the in-order
        # scalar stream) so early loads get the full bandwidth.
        XAs = [[None, None] for _ in range(BPC)]
        XBs = [[None, None] for _ in range(BPC)]
        for b in range(BPC):
            for s in range(2):
                XAs[b][s] = sb.tile([128, SH * PW], bf16,
                                    name=f"xa_{b}_{s}")
                XBs[b][s] = sb.tile([128, SH * PW], bf16,
                                    name=f"xb_{b}_{s}")

        def load_x(b, s, eng):
            base = (b * PH + s * SOFF) * PW
            eng.dma_start(XAs[b][s][:], xa[:, base:base + SH * PW])
            eng.dma_start(XBs[b][s][:], xb[:, base:base + SH * PW])

        load_x(0, 0, nc.sync)

        def conv_stage(b, g, t):
            h0 = (g * G + t) * R
            s = 1 if h0 >= SOFF else 0
            hl = h0 - s * SOFF
            xav = XAs[b][s].rearrange("p (h w) -> p h w", h=SH, w=PW)
            xbv = XBs[b][s].rearrange("p (h w) -> p h w", h=SH, w=PW)
            pc = pcp.tile([D, N], f32, name="pc")
            # taps (kh,0)+(kh,1) for kh=0,1,2 via the +1 shift
            for kh in range(3):
                nc.tensor.matmul(
                    pc[:], wa_s[:, kh * D:(kh + 1) * D],
                    xav[:, hl + kh:hl + kh + R, 0:W],
                    start=(kh == 0), stop=False)
            # taps (0,2)+(1,2) via the +PW shift
            nc.tensor.matmul(
                pc[:], wb_s[:, 0:D],
                xbv[:, hl:hl + R, 2:PW],
                start=False, stop=False)
            # tap (2,2) alone: upper weight rows are zero, upper data rows
            # only need to be finite
            nc.tensor.matmul(
                pc[:], wb_s[:, D:2 * D],
                xbv[:, hl + 2:hl + 2 + R, 2:PW],
                start=False, stop=True)
            # PSUM conv evacuation, split across Act + DVE engines
            c1 = conv1p.tile([D, N], bf16, name="c1")
            nc.vector.tensor_copy(c1[:, 0:N // 2], pc[:, 0:N // 2])
            nc.scalar.copy(c1[:, N // 2:N], pc[:, N // 2:N])
            return c1

        stg_of = {}

        def mix_stage(b, g, t, c1):
            if t == 0:
                stg_of[(b, g)] = stgp.tile([128, 2 * G * N], bf16, name="stg")
            stg = stg_of[(b, g)]
            po0 = pop.tile([128, N], f32, name="po0")
            po1 = pop.tile([128, N], f32, name="po1")
            nc.tensor.matmul(po0[:], wm_s[:, 0:128], c1[:],
                             start=True, stop=True)
            nc.tensor.matmul(po1[:], wm_s[:, 128:256], c1[:],
                             start=True, stop=True)
            nc.scalar.copy(stg[:, t * N:(t + 1) * N], po0[:])
            nc.vector.tensor_copy(
                stg[:, G * N + t * N:G * N + (t + 1) * N], po1[:])
            if t == G - 1:
                # one flat 2D store per group -> hardware DGE
                nc.scalar.dma_start(out[b * NG + g], stg[:])

        # software pipeline: the mix matmuls of tile i-1 are issued after
        # the conv matmuls of tile i, so the PE never stalls on the c1
        # PSUM->SBUF evacuation latency.
        tasks = [(b, g, t)
                 for b in range(BPC) for g in range(NG) for t in range(G)]
        # staged input-load issues: each lands well before its first use
        # (tasks 12 / 24 / 36) against a near-empty DMA backlog
        load_at = {2: (0, 1), 8: (1, 0), 20: (1, 1)}
        prev = None
        for i, task in enumerate(tasks):
            if i in load_at:
                load_x(*load_at[i], nc.scalar)
            c1 = conv_stage(*task)
            if prev is not None:
                mix_stage(*prev[0], prev[1])
            prev = (task, c1)
        mix_stage(*prev[0], prev[1])

    nc.compile()
    return nc


def _get_nc():
    if "nc" not in _NC_CACHE:
        _NC_CACHE["nc"] = _build()
    return _NC_CACHE["nc"]


def _prep_inputs(x, dictionary, lookup_coefficients, lookup_indices):
    x = np.asarray(x, dtype=np.float32)
    dic = np.asarray(dictionary, dtype=np.float32)
    coeff = np.asarray(lookup_coefficients, dtype=np.float32).reshape(O, -1)
    idx = np.asarray(lookup_indices).astype(np.int64).reshape(O, -1)

    wmat = np.zeros((O, D), np.float32)
    np.add.at(wmat, (np.arange(O)[:, None], idx), coeff)
    wm = np.ascontiguousarray(wmat.T)                     # [D, O]

    dt_ = dic.transpose(1, 0, 2, 3)                       # [cin, d, kh, kw]
    wa = np.zeros((128, 3 * D), np.float32)
    wb = np.zeros((128, 2 * D), np.float32)
    for kh in range(3):
        wa[0:64, kh * D:(kh + 1) * D] = dt_[:, :, kh, 0]
        wa[64:128, kh * D:(kh + 1) * D] = dt_[:, :, kh, 1]
    wb[0:64, 0:D] = dt_[:, :, 0, 2]
    wb[64:128, 0:D] = dt_[:, :, 1, 2]
    wb[0:64, D:2 * D] = dt_[:, :, 2, 2]                   # rows 64.. stay zero

    xpad = np.zeros((B, CIN, PH, PW), ml_dtypes.bfloat16)
    xpad[:, :, 1:H + 1, 1:W + 1] = x.astype(ml_dtypes.bfloat16)
    wa = wa.astype(ml_dtypes.bfloat16)
    wb = wb.astype(ml_dtypes.bfloat16)
    wm = wm.astype(ml_dtypes.bfloat16)

    in_maps = []
    for c in range(NCORES):
        xc = xpad[c * BPC:(c + 1) * BPC].transpose(1, 0, 2, 3).reshape(CIN, F)
        # both 128-partition planes with shifted upper halves (tails zero)
        xa = np.zeros((128, F), ml_dtypes.bfloat16)
        xb = np.zeros((128, F), ml_dtypes.bfloat16)
        xa[0:CIN] = xc
        xa[CIN:, :F - 1] = xc[:, 1:]
        xb[0:CIN] = xc
        xb[CIN:, :F - PW] = xc[:, PW:]
        in_maps.append({
            "xa": xa, "xb": xb,
            "wa": wa, "wb": wb, "wm": wm,
        })
    return in_maps


def _run(in_maps, trace=False, **kw):
    nc = _get_nc()
    return run_bass_kernel_spmd(nc, in_maps, core_ids=list(range(NCORES)),
                                trace=trace, **kw)


def _unshuffle(raw):
    # staging order [BPC*NG, 128, u*G*N + t*N + r*W + w] -> [BPC, O, H, W]
    arr = np.asarray(raw, dtype=np.float32).reshape(BPC, NG, 128, 2, G, R, W)
    return arr.transpose(0, 3, 2, 1, 4, 5, 6).reshape(BPC, O, H, W)


def kernel(x, dictionary, lookup_coefficients, lookup_indices):
    in_maps = _prep_inputs(x, dictionary, lookup_coefficients, lookup_indices)
    res = _run(in_maps)
    outs = [_unshuffle(res.results[c]["out"]) for c in range(NCORES)]
    return np.concatenate(outs, axis=0)
